# revision 18
# baseline (speedup 1.0000x reference)
"""GraphSAGE(2-layer, mean-agg) + KAN MLP + classifier on 8 TRN2 NeuronCores.

v2 design:
- One canonical per-core node order permT (total-degree desc, fake-last).
- Layer-1 aggregation: host builds a slot-aligned, edge-duplicated x stream
  (xE); device consumes it as a pure streamed GEMM with PSUM accumulation
  over neighbor ranks (no gather instructions, GPSIMD stays free).
- Everything feature-major: h1T/h2T/KAN run as lhsT-style GEMMs, no
  transposes.
- Layer-2 aggregation: z table = h1 @ W_l2 computed node-major per shard,
  AllGather'd, then big-call dma_gathers (2 int16 halves) accumulate
  node-major; per-half results round-trip DRAM in bf16 and come back via
  transpose-mode dma_gathers merged into permT feature-major layout.
- KAN + classifier in bf16, feature-major, logits written in permT order
  (host unpermutes).
"""

import math
import numpy as np
import ml_dtypes

IN_DIM, HID, KAN_Q = 256, 128, 4
P = 128
WIN = 8192          # xE stream window columns
CW = 512            # chunk width (free-dim) for GEMMs/epilogues
CALLMAX = 49 * P    # max idxs per dma_gather call
_F32 = np.float32
_BF16 = ml_dtypes.bfloat16


class CFG:
    def __init__(self, n=50000, e=800000, ncores=8, th=32768):
        self.N, self.E, self.NCORES, self.TH = n, e, ncores, th
        self.SR = n // ncores
        self.SH = ((self.SR + P - 1) // P) * P
        self.NCH = self.SH // P
        self.GTOT = ncores * self.SH
        self.NCHUNK = math.ceil(self.SH / CW)
        self.WIDTHS = [min(CW, self.SH - c * CW) for c in range(self.NCHUNK)]


def _stage16(col):
    """[L] int -> [128, L//16] int16 staged (16-wrap, replicated 8x)."""
    blk = col.reshape(-1, 16).T.astype(np.int16)
    return np.tile(blk, (8, 1))


def _slot_matrix(srcs, dpos, nslots, degmax, fill):
    """mat[pos, k] = srcs of k-th edge of slot pos (fill where none)."""
    order = np.argsort(dpos, kind="stable")
    sp, si = dpos[order], srcs[order]
    starts = np.zeros(nslots + 1, np.int64)
    np.cumsum(np.bincount(sp, minlength=nslots), out=starts[1:])
    rank = np.arange(len(sp)) - starts[sp]
    mat = np.full((nslots, max(degmax, 1)), fill, np.int64)
    if len(sp):
        mat[sp, rank] = si
    return mat


def preprocess(edge_index, cfg):
    src = np.asarray(edge_index[0], np.int64)
    dst = np.asarray(edge_index[1], np.int64)
    NC, SR, SH, TH = cfg.NCORES, cfg.SR, cfg.SH, cfg.TH
    deg = np.bincount(dst, minlength=cfg.N).astype(np.int64)

    cores = []
    for c in range(NC):
        lo = c * SR
        deg_l = np.zeros(SH, np.int64)
        deg_l[:SR] = deg[lo:lo + SR]
        permT = np.lexsort((np.arange(SH), -deg_l))
        posT = np.empty(SH, np.int64)
        posT[permT] = np.arange(SH)
        m = (dst >= lo) & (dst < lo + SR)
        cores.append(dict(deg_l=deg_l, permT=permT, posT=posT,
                          s=src[m], dpos=posT[dst[m] - lo],
                          degsort=deg_l[permT]))

    # ---- common L1 chunk schedule: K_c = max over cores of chunk-max degree
    Ks = [int(max(cc["degsort"][c * CW] for cc in cores))
          for c in range(cfg.NCHUNK)]
    # stream offsets for (chunk, b, k) -> col offset; widths per chunk
    sched = []
    off = 0
    for ci, W in enumerate(cfg.WIDTHS):
        ent = []
        for b in range(2):
            for k in range(Ks[ci]):
                ent.append(off)
                off += W
        sched.append(ent)
    SC2 = off
    SC2PAD = ((SC2 + WIN - 1) // WIN) * WIN
    NW = SC2PAD // WIN

    # per-core L1 slot matrices (in permT space)
    for cc in cores:
        degmax = max(Ks) if Ks else 1
        cc["mat1"] = _slot_matrix(cc["s"], cc["dpos"], SH, degmax, -1)

    # ---- L2 planning
    gcol = np.empty(cfg.N, np.int64)
    for c in range(NC):
        lo = c * SR
        gcol[lo:lo + SR] = c * SH + cores[c]["posT"][:SR]
    padA = 0 * SH + SR + 2          # a fake (zero) row in half A
    padB = (NC - 1) * SH + SR + 2 - TH  # fake row of last core, rel to TH

    halves = []
    for h in range(2):
        percore = []
        for c in range(NC):
            cc = cores[c]
            g = gcol[cc["s"]]
            hm = (g >= TH) if h else (g < TH)
            srel = g[hm] - (TH if h else 0)
            dloc = cc["dpos"][hm]  # positions in permT space -- recompute deg
            # local node ids for half-degree ordering
            node = cc["permT"][dloc]
            deg_h = np.bincount(node, minlength=SH)
            perm_h = np.lexsort((np.arange(SH), -deg_h))
            pos_h = np.empty(SH, np.int64)
            pos_h[perm_h] = np.arange(SH)
            percore.append(dict(srel=srel, hpos=pos_h[node], deg_h=deg_h,
                                perm_h=perm_h, pos_h=pos_h))
        degmax_h = max(int(pc["deg_h"].max()) for pc in percore)
        # common level sizes (multiples of 128)
        Ls = []
        for k in range(degmax_h):
            cnt = max(int(np.count_nonzero(pc["deg_h"] > k)) for pc in percore)
            Ls.append(((cnt + P - 1) // P) * P)
        halves.append(dict(percore=percore, Ls=Ls,
                           pad=(padB if h else padA)))

    # calls: split flat level list into calls <= CALLMAX at 128 boundaries;
    # each call: (half, nrows, [(acc_chunk, stag_chunk, nch), ...])
    calls = []
    for h in (1, 0):  # B half first, then A
        Ls = halves[h]["Ls"]
        cur = 0
        segs = []

        def flush():
            nonlocal cur, segs
            if cur:
                calls.append(dict(h=h, nrows=cur, segs=segs))
                cur, segs = 0, []
        for k, L in enumerate(Ls):
            lvl_off = 0
            while lvl_off < L:
                take = min(L - lvl_off, CALLMAX - cur)
                segs.append((lvl_off // P, cur // P, take // P))
                cur += take
                lvl_off += take
                if cur == CALLMAX:
                    flush()
        flush()

    # per-core flat idx arrays per call + merge idx
    for h in (0, 1):
        H = halves[h]
        for pc in H["percore"]:
            flat = []
            for k, L in enumerate(H["Ls"]):
                col = np.full(L, H["pad"], np.int64)
                sel = pc["deg_h"] > k
                n = int(sel.sum())
                # slots for level k: positions 0..cnt-1 in pos_h order
                mk = pc.get("matk")
                if mk is None:
                    mk = _slot_matrix(pc["srel"], pc["hpos"], SH,
                                      len(H["Ls"]), -1)
                    pc["matk"] = mk
                col[:n] = mk[:n, k]
                assert (col[:n] >= 0).all()
                flat.append(col)
            pc["flat"] = np.concatenate(flat) if flat else np.zeros(0, np.int64)

    cores_out = []
    for c in range(NC):
        cc = cores[c]
        idx_cols = []
        ofsA = ofsB = 0
        for call in calls:
            pc = halves[call["h"]]["percore"][c]
            if call["h"]:
                seg = pc["flat"][ofsB:ofsB + call["nrows"]]
                ofsB += call["nrows"]
            else:
                seg = pc["flat"][ofsA:ofsA + call["nrows"]]
                ofsA += call["nrows"]
            idx_cols.append(_stage16(seg))
        idx2 = (np.concatenate(idx_cols, axis=1) if idx_cols
                else np.zeros((P, 0), np.int16))
        # merge idx: target permT pos t (node u=permT[t]) <- pos_h[u] rows
        mgA = _stage16(halves[0]["percore"][c]["pos_h"][cc["permT"]])
        mgB = _stage16(halves[1]["percore"][c]["pos_h"][cc["permT"]])
        inv = (1.0 / np.maximum(cc["degsort"], 1.0)).astype(_F32)
        cores_out.append(dict(permT=cc["permT"], mat1=cc["mat1"],
                              idx2=idx2, mgA=mgA, mgB=mgB, invT=inv))

    plans = dict(Ks=Ks, sched=sched, SC2=SC2, SC2PAD=SC2PAD, NW=NW,
                 calls=calls, sidx2=cores_out[0]["idx2"].shape[1])
    return cores_out, plans


# ================================================================ device build
def build(cfg, plans, gelu_func="Gelu_apprx_tanh"):
    import contextlib
    import concourse.bacc as bacc
    import concourse.mybir as mybir
    from concourse.tile import TileContext
    from concourse.library_config import mlp

    f32, bf16, i16 = mybir.dt.float32, mybir.dt.bfloat16, mybir.dt.int16
    AF = mybir.ActivationFunctionType
    GELU = getattr(AF, gelu_func)
    SH, TH, GTOT, NCH = cfg.SH, cfg.TH, cfg.GTOT, cfg.NCH
    NCHUNK, WIDTHS = cfg.NCHUNK, cfg.WIDTHS
    Ks, sched, NW = plans["Ks"], plans["sched"], plans["NW"]
    SIDX2 = max(plans["sidx2"], 8)
    CALLS = plans["calls"]

    nc = bacc.Bacc(None, target_bir_lowering=False, debug=False,
                   num_swdge_queues=4)

    def din(name, shape, dt):
        return nc.dram_tensor(name, shape, dt, kind="ExternalInput")

    xE = din("xE", [P, plans["SC2PAD"]], bf16)
    xpT = din("xpT", [P, 2, SH], bf16)
    Wl1 = din("Wl1", [IN_DIM, HID], bf16)   # lhsT blocks
    Wr1 = din("Wr1", [IN_DIM, HID], bf16)
    Wl2 = din("Wl2", [HID, HID], bf16)      # rhs for zN gemm
    Wr2 = din("Wr2", [HID, HID], bf16)      # lhsT for root2
    Wk1 = din("Wk1", [HID, HID * KAN_Q], bf16)
    Wk2 = din("Wk2", [HID * KAN_Q, HID], bf16)
    Wc = din("Wc", [HID, 1], bf16)
    bl1 = din("bl1", [P, 1], f32)
    bl2 = din("bl2", [P, 1], f32)
    bk1 = din("bk1", [P, KAN_Q], f32)
    bk2 = din("bk2", [P, 1], f32)
    bc = din("bc", [1, 1], f32)
    invb = din("invb", [P, SH], bf16)
    idx2 = din("idx2", [P, SIDX2], i16)
    mgA = din("mgA", [P, SH // 16], i16)
    mgB = din("mgB", [P, SH // 16], i16)
    logits = nc.dram_tensor("logits", [SH], f32, kind="ExternalOutput")

    with TileContext(nc) as tc, contextlib.ExitStack() as ctx:
        dram = ctx.enter_context(tc.tile_pool(name="dram", bufs=1, space="DRAM"))
        const = ctx.enter_context(tc.tile_pool(name="const", bufs=1))
        xe = ctx.enter_context(tc.tile_pool(name="xe", bufs=2))
        gp = ctx.enter_context(tc.tile_pool(name="gath", bufs=2))
        accp = ctx.enter_context(tc.tile_pool(name="accs", bufs=1))
        work = ctx.enter_context(tc.tile_pool(name="work", bufs=2))
        ps = ctx.enter_context(tc.tile_pool(name="ps", bufs=2, space="PSUM"))
        ps2 = ctx.enter_context(tc.tile_pool(name="ps2", bufs=2, space="PSUM"))

        zloc = dram.tile([SH, HID], bf16)
        zfull = dram.tile([GTOT, HID], bf16, addr_space="Shared")
        accAd = dram.tile([SH, HID], bf16)
        accBd = dram.tile([SH, HID], bf16)

        nc.gpsimd.load_library(mlp)

        def ld(dr, shape, dt, tag):
            t = const.tile(shape, dt, tag=tag)
            nc.sync.dma_start(out=t[:], in_=dr[:])
            return t

        wl1_t = [ld(Wl1[k * P:(k + 1) * P, :], [P, HID], bf16, f"wl1{k}")
                 for k in range(2)]
        wr1_t = [ld(Wr1[k * P:(k + 1) * P, :], [P, HID], bf16, f"wr1{k}")
                 for k in range(2)]
        wl2_t = ld(Wl2, [HID, HID], bf16, "wl2")
        wr2_t = ld(Wr2, [HID, HID], bf16, "wr2")
        wk1_t = ld(Wk1, [HID, HID * KAN_Q], bf16, "wk1")
        wk2_t = [ld(Wk2[k * P:(k + 1) * P, :], [P, HID], bf16, f"wk2{k}")
                 for k in range(KAN_Q)]
        wc_t = ld(Wc, [HID, 1], bf16, "wc")
        bl1_t = ld(bl1, [P, 1], f32, "bl1")
        bl2_t = ld(bl2, [P, 1], f32, "bl2")
        bk1_t = ld(bk1, [P, KAN_Q], f32, "bk1")
        bk2_t = ld(bk2, [P, 1], f32, "bk2")
        bc_t = ld(bc, [1, 1], f32, "bc")
        invb_t = ld(invb, [P, SH], bf16, "invb")
        mgA_t = ld(mgA, [P, SH // 16], i16, "mgA")
        mgB_t = ld(mgB, [P, SH // 16], i16, "mgB")

        h1T = accp.tile([P, SH], bf16, tag="h1T")

        # ---------------- L1: streamed xE GEMM + root + epilogue
        wtiles = {}

        def win_tile(w):
            if w not in wtiles:
                t = xe.tile([P, WIN], bf16, tag="xe")
                nc.sync.dma_start(out=t[:], in_=xE[:, w * WIN:(w + 1) * WIN])
                wtiles[w] = t
            return wtiles[w]

        for ci in range(NCHUNK):
            W = WIDTHS[ci]
            cs = ci * CW
            K = Ks[ci]
            psR = ps.tile([P, CW], mybir.dt.float32, tag="mmR")
            xpc = xe.tile([P, 2, CW], bf16, tag="xp")
            nc.sync.dma_start(out=xpc[:, :, :W], in_=xpT[:, :, cs:cs + W])
            for b in range(2):
                nc.tensor.matmul(psR[:, :W], wr1_t[b][:], xpc[:, b, :W],
                                 start=(b == 0), stop=(b == 1))
            if K > 0:
                psA = ps.tile([P, CW], mybir.dt.float32, tag="mmA")
                nmm = 2 * K
                i = 0
                for b in range(2):
                    for k in range(K):
                        off = sched[ci][b * K + k]
                        t = win_tile(off // WIN)
                        o = off % WIN
                        nc.tensor.matmul(psA[:, :W], wl1_t[b][:], t[:, o:o + W],
                                         start=(i == 0), stop=(i == nmm - 1))
                        i += 1
                t1 = work.tile([P, CW], f32, tag="t1")
                nc.vector.tensor_mul(t1[:, :W], psA[:, :W], invb_t[:, cs:cs + W])
                nc.vector.tensor_add(t1[:, :W], t1[:, :W], psR[:, :W])
                nc.scalar.activation(h1T[:, cs:cs + W], t1[:, :W], AF.Relu,
                                     bias=bl1_t[:])
            else:
                nc.scalar.activation(h1T[:, cs:cs + W], psR[:, :W], AF.Relu,
                                     bias=bl1_t[:])
            # z-table chunks for this L1 chunk (node-major), AG input
            if ci == NCHUNK - 1:
                nc.vector.memset(h1T[:, cfg.SR:SH], 0.0)
            nb = W // P
            zb4 = work.tile([P, 4, HID], bf16, tag="zb4")
            for j in range(nb):
                cc = ci * 4 + j
                psZ = ps2.tile([P, CW], mybir.dt.float32, tag="mmK")
                nc.tensor.matmul(psZ[:, :HID], h1T[:, cc * P:(cc + 1) * P],
                                 wl2_t[:], start=True, stop=True)
                nc.scalar.activation(zb4[:, j, :], psZ[:, :HID], AF.Copy)
            nc.sync.dma_start(
                out=zloc[ci * 4 * P:(ci * 4 + nb) * P, :].rearrange(
                    "(c p) f -> p c f", p=P),
                in_=zb4[:, :nb, :])

        nc.gpsimd.collective_compute(
            "AllGather", mybir.AluOpType.bypass,
            replica_groups=[list(range(cfg.NCORES))],
            ins=[zloc.opt()], outs=[zfull.opt()])

        # ---------------- root2 precompute (tensor idle during gathers)
        r2T = accp.tile([P, SH], bf16, tag="r2T")
        for ci in range(NCHUNK):
            W = WIDTHS[ci]
            cs = ci * CW
            psR = ps.tile([P, CW], mybir.dt.float32, tag="mmR")
            nc.tensor.matmul(psR[:, :W], wr2_t[:], h1T[:, cs:cs + W],
                             start=True, stop=True)
            nc.scalar.activation(r2T[:, cs:cs + W], psR[:, :W], AF.Copy)

        # ---------------- L2 gathers (B half first, then A), node-major accs
        accH = accp.tile([P, NCH, HID], f32, tag="accH")
        col = 0
        cur_h = None
        for call in CALLS:
            h, nrows = call["h"], call["nrows"]
            if h != cur_h:
                if cur_h is not None:
                    # finalize previous half: convert + store
                    accb = accp.tile([P, NCH, HID], bf16, tag="accb")
                    nc.vector.tensor_copy(accb[:], accH[:])
                    ddst = accBd if cur_h == 1 else accAd
                    nc.sync.dma_start(
                        out=ddst[:].rearrange("(c p) f -> p c f", p=P),
                        in_=accb[:])
                    accH = accp.tile([P, NCH, HID], f32, tag="accH")
                nc.vector.memset(accH[:], 0.0)
                cur_h = h
            s = nrows // 16
            idxc = work.tile([P, CALLMAX // 16], i16, tag="idxc")
            nc.sync.dma_start(out=idxc[:, :s], in_=idx2[:, col:col + s])
            g = gp.tile([P, CALLMAX // P, HID], bf16, tag="g")
            src_ap = zfull[TH:GTOT, :] if h else zfull[0:TH, :]
            nc.gpsimd.dma_gather(g[:, :nrows // P, :], src_ap,
                                 idxc[:, :s], nrows, nrows, HID,
                                 single_packet=False, queue_num=col % 4)
            col += s
            for (a, sg, n) in call["segs"]:
                nc.vector.tensor_add(
                    accH[:, a:a + n, :].rearrange("p c f -> p (c f)"),
                    accH[:, a:a + n, :].rearrange("p c f -> p (c f)"),
                    g[:, sg:sg + n, :].rearrange("p c f -> p (c f)"))
        # finalize last half (A)
        accb = accp.tile([P, NCH, HID], bf16, tag="accb")
        nc.vector.tensor_copy(accb[:], accH[:])
        nc.sync.dma_start(out=accAd[:].rearrange("(c p) f -> p c f", p=P),
                          in_=accb[:])

        # ---------------- merge to feature-major permT + epilogue 2
        gA = accp.tile([P, 1, SH], bf16, tag="gA")
        gB = accp.tile([P, 1, SH], bf16, tag="gB")
        nc.gpsimd.dma_gather(gA[:], accAd[:], mgA_t[:], SH, SH, HID,
                             transpose=True, single_packet=False, queue_num=0)
        nc.gpsimd.dma_gather(gB[:], accBd[:], mgB_t[:], SH, SH, HID,
                             transpose=True, single_packet=False, queue_num=1)
        h2T = accp.tile([P, SH], bf16, tag="h2T")
        for ci in range(NCHUNK):
            W = WIDTHS[ci]
            cs = ci * CW
            t1 = work.tile([P, CW], f32, tag="t1")
            nc.vector.tensor_add(t1[:, :W], gA[:, 0, cs:cs + W],
                                 gB[:, 0, cs:cs + W])
            nc.vector.tensor_mul(t1[:, :W], t1[:, :W], invb_t[:, cs:cs + W])
            nc.vector.tensor_add(t1[:, :W], t1[:, :W], r2T[:, cs:cs + W])
            nc.scalar.activation(h2T[:, cs:cs + W], t1[:, :W], AF.Relu,
                                 bias=bl2_t[:])

        # ---------------- KAN + classifier (feature-major)
        for ci in range(NCHUNK):
            W = WIDTHS[ci]
            cs = ci * CW
            g1 = work.tile([P, KAN_Q, CW], bf16, tag="g1")
            for j in range(KAN_Q):
                psK = ps2.tile([P, CW], mybir.dt.float32, tag="mmK")
                nc.tensor.matmul(psK[:, :W], wk1_t[:, j * P:(j + 1) * P],
                                 h2T[:, cs:cs + W], start=True, stop=True)
                nc.scalar.activation(g1[:, j, :W], psK[:, :W], GELU,
                                     bias=bk1_t[:, j:j + 1])
            psK2 = ps2.tile([P, CW], mybir.dt.float32, tag="mmK")
            for j in range(KAN_Q):
                nc.tensor.matmul(psK2[:, :W], wk2_t[j][:], g1[:, j, :W],
                                 start=(j == 0), stop=(j == KAN_Q - 1))
            g2 = work.tile([P, CW], bf16, tag="g2")
            nc.scalar.activation(g2[:, :W], psK2[:, :W], GELU, bias=bk2_t[:])
            psC = ps.tile([1, CW], mybir.dt.float32, tag="mmC")
            nc.tensor.matmul(psC[:, :W], wc_t[:], g2[:, :W],
                             start=True, stop=True)
            lgc = work.tile([1, CW], f32, tag="lgc")
            nc.vector.tensor_scalar_add(lgc[:, :W], psC[:, :W],
                                        bc_t[0:1, 0:1])
            nc.sync.dma_start(out=logits[None, cs:cs + W], in_=lgc[:, :W])

    nc.compile()
    return nc


# ================================================================ entry point
_CACHE = {}


def _run(cfg, x, edge_index, W_l1, b_l1, W_r1, W_l2, b_l2, W_r2,
         W_k1, b_k1, W_k2, b_k2, W_c, b_c, gelu_func="Gelu_apprx_tanh",
         trace=False, tmpdir=None, _res_out=None):
    from concourse.bass_utils import run_bass_kernel_spmd

    x = np.asarray(x, _F32)
    cores, plans = preprocess(edge_index, cfg)
    key = (cfg.N, cfg.E, cfg.NCORES, plans["SC2PAD"], plans["sidx2"],
           tuple(plans["Ks"]), gelu_func,
           tuple((c["h"], c["nrows"]) for c in plans["calls"]))
    if key not in _CACHE:
        _CACHE[key] = build(cfg, plans, gelu_func)
    nc = _CACHE[key]

    # x transposed with a zero column at index N (for -1 slots)
    xT0 = np.zeros((IN_DIM, cfg.N + 1), _F32)
    xT0[:, :cfg.N] = x.T
    xT0 = xT0.astype(_BF16)

    common = dict(
        Wl1=np.asarray(W_l1, _F32).astype(_BF16),
        Wr1=np.asarray(W_r1, _F32).astype(_BF16),
        Wl2=np.asarray(W_l2, _F32).astype(_BF16),
        Wr2=np.asarray(W_r2, _F32).astype(_BF16),
        Wk1=np.asarray(W_k1, _F32).astype(_BF16),
        Wk2=np.asarray(W_k2, _F32).astype(_BF16),
        Wc=np.asarray(W_c, _F32).astype(_BF16),
        bl1=np.asarray(b_l1, _F32).reshape(P, 1),
        bl2=np.asarray(b_l2, _F32).reshape(P, 1),
        bk1=np.ascontiguousarray(np.asarray(b_k1, _F32).reshape(KAN_Q, P).T),
        bk2=np.asarray(b_k2, _F32).reshape(P, 1),
        bc=np.asarray(b_c, _F32).reshape(1, 1),
    )

    in_maps = []
    for c in range(cfg.NCORES):
        cc = cores[c]
        lo = c * cfg.SR
        permT = cc["permT"]

        # xE stream: per chunk, b-outer, k-level columns of x
        xEa = np.zeros((P, plans["SC2PAD"]), _BF16)
        mat = cc["mat1"]
        for ci, W in enumerate(cfg.WIDTHS):
            K = plans["Ks"][ci]
            if K == 0:
                continue
            srcs = mat[ci * CW:ci * CW + W, :K].T.reshape(-1).copy()
            srcs[srcs < 0] = cfg.N
            cols = xT0[:, srcs]  # [256, K*W]
            for b in range(2):
                for k in range(K):
                    off = plans["sched"][ci][b * K + k]
                    xEa[:, off:off + W] = cols[b * P:(b + 1) * P,
                                               k * W:(k + 1) * W]

        # root stream: x rows of permT order (fake -> zero)
        nodes = np.minimum(permT, cfg.SR - 1) + lo
        xp = x[nodes].copy()
        xp[permT >= cfg.SR] = 0.0
        xpTa = np.ascontiguousarray(xp.T).astype(_BF16).reshape(2, P, cfg.SH)
        xpTa = np.ascontiguousarray(xpTa.transpose(1, 0, 2))

        m = dict(common)
        m.update(
            xE=xEa, xpT=xpTa,
            invb=np.tile(cc["invT"].reshape(1, cfg.SH), (P, 1)).astype(_BF16),
            idx2=_pad_cols(cc["idx2"], max(plans["sidx2"], 8)),
            mgA=cc["mgA"], mgB=cc["mgB"],
        )
        in_maps.append(m)

    res = run_bass_kernel_spmd(nc, in_maps, core_ids=list(range(cfg.NCORES)),
                               trace=trace, tmpdir=tmpdir)
    if _res_out is not None:
        _res_out.append(res)
    out = np.zeros(cfg.N, _F32)
    for c in range(cfg.NCORES):
        lo = c * cfg.SR
        shard = res.results[c]["logits"]
        permT = cores[c]["permT"]
        real = permT < cfg.SR
        out[lo + permT[real]] = shard[real]
    return out


def _pad_cols(a, w):
    out = np.zeros((P, w), a.dtype)
    out[:, :a.shape[1]] = a
    return out


def kernel(**inputs):
    cfg = CFG()
    return _run(cfg, **inputs)
